# revision 63
# baseline (speedup 1.0000x reference)
"""Trainium2 Bass kernel for DiagonalKernelAverageV2.

Math: for each (b, ch) image X [512, 512] and each of 4 corners, the output
at index i is the mean over the L-shaped shell of the i-th nested corner
square:  shell[i] = d[i] - d[i-1],  d[i] = sum of the (i+1)x(i+1) corner
window,  counts[i] = 2i+1.

Only two shell families are computed directly (top-left and top-right); the
bottom corners follow from row/col totals:
    shell_tl[i] = sum_{c<=i} X[i,c] + sum_{r<i}  X[r,i]
    shell_tr[i] = sum_{c>=511-i} X[i,c] + sum_{r<i} X[r,511-i]
    shell_br[i] = S[511-i] + ST[511-i] - shell_tl[511-i]
    shell_bl[i] = S[511-i] + ST[i]     - shell_tr[511-i]
(S = row sums, ST = col sums.)

Inputs are fed to the device as fp16 and outputs returned as fp16
(quantization rel-err ~1e-3, well under the 2e-2 gate); this halves HBM
traffic — the dominant resource — and runs every PE matmul at the
1-cycle/row rate.  The kernel is DMA-bound: the input stream (16 MB/core at
~332 GB/s in the cost model) is kept back-to-back via pair-granularity
loads, with the first pair issued before the (packed, 2-DMA) constants.

Per-core layout: batch-sharded (4 batches x 8 channels per core).  Each image
is 4 row-tiles [128, 512] in X8; per pair of images a PP tile holds 8 masked
product blocks (P1 = strict-upper-masked diagonal blocks, P2rev =
strict-upper-masked reversed antidiagonal blocks).  Per-pair work by engine:
  - GpSimd: both masked products; tail of the block-sum tree; per-pair
    assembly and output weighting (into the fp16 o_full tile).
  - VectorE: first two levels of the pairwise-add block-sum tree (fp16 2x
    tensor_tensor), half of level 3, psumT->TQ copies.  The tree covers X and
    PP blocks together -> B24 = 16 block sums + RSsu + RS2su per image.
  - TensorE: per-image one PSUM accum group (4 X-tile matmuls with
    prefix-weight columns + 2 matmuls over P1/P2rev -> CPfx/ST/CS1/CS2);
    transposes (deferred one pair so PE never waits on staging) move
    column-indexed rows onto partitions.
  - ScalarE: psumQ->SBUF staging copies (forward + reversed).
A dependency chain on PE keeps accum groups and transposes from
interleaving.  One fp16 output DMA at the end; bottom-corner outputs are
written in source order, flipped and upcast to fp32 on the host.
"""

import numpy as np

SIZE = 512
NT = 4  # row tiles per image
NCH = 8  # channels per batch
NB_CORE = 4  # batches per core
N_CORES = 8
NQ = 10  # transposed quantity cols per tile: 6 fwd + 4 reversed
NPAIR = NCH // 2


def build_nc():
    import concourse.bass as bass
    import concourse.bacc as bacc
    import concourse.mybir as mybir
    from concourse.tile import TileContext

    f32 = mybir.dt.float32
    f16 = mybir.dt.float16
    nc = bacc.Bacc()

    x = nc.dram_tensor("x", [NB_CORE, NCH, SIZE, SIZE], f16, kind="ExternalInput")
    # packed consts: ch (f16): msu [0:1024] | vw [1024:1060];
    # cf (f32): wg [0:32] | wrevg [32:64] | eye [64:72] (rows 0-7)
    ch_d = nc.dram_tensor("ch", [128, 1060], f16, kind="ExternalInput")
    cf_d = nc.dram_tensor("cf", [128, 72], f32, kind="ExternalInput")
    out = nc.dram_tensor("out", [NB_CORE, SIZE, 4 * NCH], f16, kind="ExternalOutput")

    ADD = mybir.AluOpType.add
    MULT = mybir.AluOpType.mult
    SUB = mybir.AluOpType.subtract

    with TileContext(nc) as tc, nc.allow_low_precision(reason="fp16 pipeline"):
        with (
            tc.tile_pool(name="consts", bufs=1) as consts,
            tc.tile_pool(name="xs", bufs=2) as xpool,
            tc.tile_pool(name="tree", bufs=3) as trpool,
            tc.tile_pool(name="tin", bufs=3) as tinpool,
            tc.tile_pool(name="perb", bufs=3) as bpool,
            tc.tile_pool(name="small", bufs=3) as spool,
            tc.tile_pool(name="psq", bufs=4, space="PSUM") as psq,
            tc.tile_pool(name="pst", bufs=4, space="PSUM") as pst,
        ):
            # batch-0 input DMAs first so the DMA device starts on the
            # critical data immediately; consts follow
            X8s = []
            for b in range(NB_CORE):
                X8b = xpool.tile(
                    [128, NCH, NT, SIZE], f16, tag=f"x8_{b % 2}", name=f"x8b{b}"
                )
                X8s.append(X8b)
            nc.sync.dma_start(
                out=X8s[0][:, 0:2],
                in_=x[0, 0:2].rearrange("i (t p) c -> p i t c", p=128),
            )
            ch = consts.tile([128, 1060], f16)
            nc.sync.dma_start(out=ch, in_=ch_d[:])
            cf = consts.tile([128, 72], f32)
            nc.sync.dma_start(out=cf, in_=cf_d[:])
            msu = ch[:, 0:1024]
            vw = ch[:, 1024:1060]
            wg = cf[:, 0:32].rearrange("p (g t) -> p g t", t=NT)
            wrevg = cf[:, 32:64].rearrange("p (g t) -> p g t", t=NT)
            eye = cf[:, 64:72]
            for gp in range(1, NPAIR):
                nc.sync.dma_start(
                    out=X8s[0][:, 2 * gp : 2 * gp + 2],
                    in_=x[0, 2 * gp : 2 * gp + 2].rearrange(
                        "i (t p) c -> p i t c", p=128
                    ),
                )
            msu8 = msu.rearrange("p (i t c) -> p i t c", i=2, c=128)

            o_full = consts.tile([128, NB_CORE, NT, 4, NCH], f16)

            from concourse.bass import _add_dep_helper

            def emit_pair_tail(entry, B24, TQ, o_all, prev_pe_last):
                """Transposes + TQ staging + per-pair assembly for one pair."""
                gp, items = entry
                for g, Tin, TinB in items:
                    psumT = pst.tile([128, NT * NQ], f32)
                    for t in range(NT):
                        tr = nc.tensor.transpose(
                            psumT[:, NQ * t : NQ * t + 6],
                            in_=Tin[0:6, 128 * t : 128 * (t + 1)],
                            identity=eye[0:6, 0:6],
                        )
                        if t == 0:
                            _add_dep_helper(
                                tr.ins, prev_pe_last.ins, sync=False,
                                reason="PE group ordering",
                            )
                        prev_pe_last = nc.tensor.transpose(
                            psumT[:, NQ * t + 6 : NQ * t + 10],
                            in_=TinB[0:4, 128 * t : 128 * (t + 1)],
                            identity=eye[0:4, 0:4],
                        )
                    nc.vector.tensor_copy(
                        TQ[:, g].rearrange("p t q -> p (t q)"), psumT[:, :]
                    )

                # ---- per-pair assembly on GpSimd ([128, (2g), (t)] ops) ----
                def bg_ap(base, tstep):
                    return bass.AP(
                        tensor=B24.tensor,
                        offset=B24[:, 0, 0:1].offset + 2 * gp * 24 + base,
                        ap=[B24[:, 0, 0:1].ap[0]] + [[24, 2], [tstep, NT]],
                    )

                def tq_ap(base, tstep, nt=NT):
                    return bass.AP(
                        tensor=TQ.tensor,
                        offset=TQ[:, 0, 0, 0:1].offset + 2 * gp * NT * NQ + base,
                        ap=[TQ[:, 0, 0, 0:1].ap[0]] + [[NT * NQ, 2], [tstep, nt]],
                    )

                PI = spool.tile([128, 2, 5, NT], f32, tag="pi")

                def pi_ap(base, tstep, nt=NT):
                    return bass.AP(
                        tensor=PI.tensor,
                        offset=PI[:, 0, 0, 0:1].offset + base,
                        ap=[PI[:, 0, 0, 0:1].ap[0]] + [[20, 2], [tstep, nt]],
                    )

                nc.gpsimd.memset(PI[:, :, 0, :], 0.0)
                nc.gpsimd.tensor_copy(PI[:, :, 1, :], bg_ap(0, 4))
                for m in range(2, 5):
                    nc.gpsimd.tensor_tensor(
                        PI[:, :, m, :], PI[:, :, m - 1, :], bg_ap(m - 1, 4),
                        op=ADD,
                    )

                sh_tl = spool.tile([128, 2, NT], f32, tag="shtl")
                sh_tr = spool.tile([128, 2, NT], f32, tag="shtr")
                # shell_tl = B[t][t] - RSsu + PI[m=t] + CPfx[m=t] + CS1
                nc.gpsimd.tensor_tensor(sh_tl, bg_ap(0, 5), bg_ap(16, 1), op=SUB)
                nc.gpsimd.tensor_tensor(sh_tl, sh_tl, pi_ap(0, 5), op=ADD)
                nc.gpsimd.tensor_tensor(
                    sh_tl[:, :, 1:4], sh_tl[:, :, 1:4], tq_ap(NQ, NQ + 1, 3),
                    op=ADD,
                )
                nc.gpsimd.tensor_tensor(sh_tl, sh_tl, tq_ap(4, NQ), op=ADD)
                # shell_tr = B[t][3-t] - RS2su + S - PI[m=4-t] + CPfxRev[m=t] + CS2
                nc.gpsimd.tensor_tensor(sh_tr, bg_ap(3, 3), bg_ap(20, 1), op=SUB)
                nc.gpsimd.tensor_tensor(sh_tr, sh_tr, pi_ap(16, 1), op=ADD)
                nc.gpsimd.tensor_tensor(sh_tr, sh_tr, pi_ap(16, -3), op=SUB)
                nc.gpsimd.tensor_tensor(
                    sh_tr[:, :, 1:4], sh_tr[:, :, 1:4],
                    tq_ap(NQ + 6, NQ + 1, 3), op=ADD,
                )
                nc.gpsimd.tensor_tensor(sh_tr, sh_tr, tq_ap(5, NQ), op=ADD)
                # br (src order): u = ST - shell_tl + S ; bl: v = STrev - shell_tr + S
                u = spool.tile([128, 2, NT], f32, tag="u")
                v = spool.tile([128, 2, NT], f32, tag="v")
                nc.gpsimd.tensor_tensor(u, tq_ap(3, NQ), sh_tl, op=SUB)
                nc.gpsimd.tensor_tensor(u, u, pi_ap(16, 1), op=ADD)
                nc.gpsimd.tensor_tensor(v, tq_ap(9, NQ), sh_tr, op=SUB)
                nc.gpsimd.tensor_tensor(v, v, pi_ap(16, 1), op=ADD)
                for ci, (src, wt) in enumerate(
                    [(sh_tl, wg), (sh_tr, wg), (v, wrevg), (u, wrevg)]
                ):
                    nc.gpsimd.tensor_tensor(
                        o_all[:, :, ci, 2 * gp : 2 * gp + 2],
                        src.rearrange("p g t -> p t g"),
                        wt[:, 2 * gp : 2 * gp + 2].rearrange("p g t -> p t g"),
                        op=MULT,
                    )
                return prev_pe_last

            prev_pe_last = None
            for b in range(NB_CORE):
                # B24[p, g, k]: k=4t+j -> block sum B[t][j]; k=16+t -> RSsu[t];
                # k=20+t -> RS2su[t]
                B24 = bpool.tile([128, NCH, 24], f32, tag="b24")
                TQ = bpool.tile([128, NCH, NT, NQ], f32, tag="tq")
                o_all = o_full[:, b]

                # input DMAs at pair granularity (finer pipelining, less
                # head-of-line latency; APs merge to 3D); batch 0 was issued
                # up front
                staged = []
                X8 = X8s[b]
                if b > 0:
                    for gp in range(NPAIR):
                        nc.sync.dma_start(
                            out=X8[:, 2 * gp : 2 * gp + 2],
                            in_=x[b, 2 * gp : 2 * gp + 2].rearrange(
                                "i (t p) c -> p i t c", p=128
                            ),
                        )

                for gp in range(NPAIR):
                    Xpr = X8[:, 2 * gp : 2 * gp + 2]  # [128, 2, NT, SIZE]
                    x0 = Xpr[:, 0, 0, 0:1]

                    def blk_ap(base, tstep, cstep=1, coff=0):
                        # [p][i(2)][t(4)][c(128)] over the pair
                        return bass.AP(
                            tensor=X8.tensor,
                            offset=x0.offset + base * 128 + coff,
                            ap=[x0.ap[0]]
                            + [[NT * SIZE, 2], [tstep * 128, NT], [cstep, 128]],
                        )

                    # masked products on GpSimd -> PP (blocks 0-3 = P1,
                    # 4-7 = P2rev, per image)
                    PP = xpool.tile([128, 2, 8, 128], f16, tag="pp")
                    nc.gpsimd.tensor_tensor(
                        PP[:, :, 0:4, :], blk_ap(0, 5), msu8, op=MULT
                    )
                    nc.gpsimd.tensor_tensor(
                        PP[:, :, 4:8, :],
                        blk_ap(3, 3, cstep=-1, coff=127),
                        msu8,
                        op=MULT,
                    )

                    # block row sums: fp16 2x pairwise-add tree; first two
                    # levels on VectorE, tail on GpSimd.  T* blocks 0-15 = X,
                    # 16-23 = PP.
                    T1 = trpool.tile([128, 2, 24, 64], f16, tag="t1")
                    T2 = trpool.tile([128, 2, 24, 32], f16, tag="t2")
                    T3 = trpool.tile([128, 2, 24, 16], f16, tag="t3")
                    T4 = trpool.tile([128, 2, 24, 8], f16, tag="t4")
                    T5 = trpool.tile([128, 2, 24, 4], f16, tag="t5")
                    T6 = trpool.tile([128, 2, 24, 2], f16, tag="t6")
                    Xblk = Xpr.rearrange("p i t (j c) -> p i (t j) c", c=128)
                    nc.vector.tensor_tensor(
                        T1[:, :, 0:16, :], Xblk[:, :, :, 0:64],
                        Xblk[:, :, :, 64:128], op=ADD,
                    )
                    nc.vector.tensor_tensor(
                        T1[:, :, 16:24, :], PP[:, :, :, 0:64],
                        PP[:, :, :, 64:128], op=ADD,
                    )
                    nc.vector.tensor_tensor(
                        T2, T1[:, :, :, 0:32], T1[:, :, :, 32:64], op=ADD
                    )
                    nc.vector.tensor_tensor(
                        T3[:, 0], T2[:, 0, :, 0:16], T2[:, 0, :, 16:32], op=ADD
                    )
                    nc.gpsimd.tensor_tensor(
                        T3[:, 1], T2[:, 1, :, 0:16], T2[:, 1, :, 16:32], op=ADD
                    )
                    nc.gpsimd.tensor_tensor(
                        T4, T3[:, :, :, 0:8], T3[:, :, :, 8:16], op=ADD
                    )
                    nc.gpsimd.tensor_tensor(
                        T5, T4[:, :, :, 0:4], T4[:, :, :, 4:8], op=ADD
                    )
                    nc.gpsimd.tensor_tensor(
                        T6, T5[:, :, :, 0:2], T5[:, :, :, 2:4], op=ADD
                    )
                    nc.gpsimd.tensor_tensor(
                        B24[:, 2 * gp : 2 * gp + 2],
                        T6[:, :, :, 0],
                        T6[:, :, :, 1],
                        op=ADD,
                    )

                    # column-side quantities on PE: per image one accum group;
                    # rows 0-2: CPfx[1..3], 3: ST, 4: colsum(P1), 5: colsum(P2rev).
                    # Transposes run one pair behind the matmul groups so PE
                    # never stalls on the PSUM->SBUF staging.
                    pair_staged = []
                    for i in (0, 1):
                        XPi = Xpr[:, i].rearrange("p a b -> p (a b)")
                        PPi = PP[:, i].rearrange("p a b -> p (a b)")
                        psumQ = psq.tile([6, SIZE], f32)
                        for t in range(NT):
                            mm = nc.tensor.matmul(
                                psumQ[0:6, :],
                                lhsT=vw[:, 6 * t : 6 * t + 6],
                                rhs=XPi[:, 512 * t : 512 * (t + 1)],
                                start=(t == 0),
                                stop=False,
                            )
                            # keep PE program order: no transpose-mode matmul
                            # and no other accum group may interleave here
                            if t == 0 and prev_pe_last is not None:
                                _add_dep_helper(
                                    mm.ins, prev_pe_last.ins, sync=False,
                                    reason="PE group ordering",
                                )
                        nc.tensor.matmul(
                            psumQ[0:6, :], lhsT=vw[:, 24:30],
                            rhs=PPi[:, 0:512], start=False, stop=False,
                        )
                        prev_pe_last = nc.tensor.matmul(
                            psumQ[0:6, :], lhsT=vw[:, 30:36],
                            rhs=PPi[:, 512:1024], start=False, stop=True,
                        )
                        # staging on ScalarE overlaps later matmul groups
                        Tin = tinpool.tile([6, SIZE], f32, tag=f"tin{i}")
                        TinB = tinpool.tile([4, SIZE], f32, tag=f"tinb{i}")
                        nc.scalar.copy(Tin[0:6, :], psumQ[0:6, :])
                        nc.scalar.copy(TinB[0:4, :], psumQ[0:4, ::-1])
                        pair_staged.append((2 * gp + i, Tin, TinB))
                    if staged:
                        prev_pe_last = emit_pair_tail(
                            staged.pop(), B24, TQ, o_all, prev_pe_last
                        )
                    staged.append((gp, pair_staged))

                prev_pe_last = emit_pair_tail(
                    staged.pop(), B24, TQ, o_all, prev_pe_last
                )
            nc.sync.dma_start(
                out=out.rearrange("b (t p) c -> p b t c", p=128),
                in_=o_full.rearrange("p b t c g -> p b t (c g)"),
            )
    nc.compile()
    return nc


def make_consts():
    r = np.arange(128)
    msu = np.tile((r[None, :] > r[:, None]).astype(np.float16), (1, 8))  # [c > r]
    vw = np.zeros((128, 36), np.float16)
    for t in range(NT):
        for m in range(3):
            vw[:, 6 * t + m] = 1.0 if t < m + 1 else 0.0  # CPfx[m+1]
        vw[:, 6 * t + 3] = 1.0  # ST
    vw[:, 24 + 4] = 1.0  # colsum(P1) -> row 4
    vw[:, 30 + 5] = 1.0  # colsum(P2rev) -> row 5
    ch = np.concatenate([msu, vw], axis=1)  # [128, 1060] f16
    i_pt = (r[:, None] + 128 * np.arange(NT)[None, :]).astype(np.float64)
    w_pt = (1.0 / (2 * i_pt + 1)).astype(np.float32)  # [128, NT]
    wrev_pt = (1.0 / (1023.0 - 2 * i_pt)).astype(np.float32)
    wg = np.tile(w_pt[:, None, :], (1, NCH, 1)).reshape(128, 32)
    wrevg = np.tile(wrev_pt[:, None, :], (1, NCH, 1)).reshape(128, 32)
    eye = np.zeros((128, 8), np.float32)
    eye[:8, :8] = np.eye(8)
    cf = np.concatenate([wg, wrevg, eye], axis=1).astype(np.float32)  # [128, 72]
    return dict(ch=ch.astype(np.float16), cf=cf)


_NC = None


def _get_nc():
    global _NC
    if _NC is None:
        _NC = build_nc()
    return _NC


def kernel(x: np.ndarray) -> np.ndarray:
    from concourse.bass_utils import run_bass_kernel_spmd

    x = np.asarray(x, dtype=np.float32).astype(np.float16)
    B = x.shape[0]
    consts = make_consts()
    per_core = B // N_CORES
    assert per_core == NB_CORE
    in_maps = [
        {"x": x[c * per_core : (c + 1) * per_core], **consts}
        for c in range(N_CORES)
    ]
    nc = _get_nc()
    res = run_bass_kernel_spmd(nc, in_maps, core_ids=list(range(N_CORES)))
    outs = []
    for r in res.results:
        o = r["out"].astype(np.float32)  # [NB_CORE, 512, 4*NCH]
        o[:, :, 2 * NCH :] = o[:, ::-1, 2 * NCH :]
        outs.append(o)
    return np.concatenate(outs, axis=0)
